# revision 24
# baseline (speedup 1.0000x reference)
"""Trainium2 Bass kernel for ragged bmm2 (attention probs @ V, grouped GEMM).

Problem: 32 ragged sequences, lengths s_i = 128 + 12*i (128..500), 16 heads,
embed 64.  batch1 = packed per-(seq,head) [s,s] prob blocks (fp32, ~227MB),
batch2 = packed V [ntokens, 16*64].  out[q,h,e] = sum_k P[h,q,k] V[k,h,e].

Sharding: head-parallel.  Core c handles heads (2c, 2c+1) for ALL sequences.

v3 design (memory-roofline oriented):
 - host pre-transposes P into PT[k, q] (bf16) so the device does no
   transposes; PT is the *moving* operand (N=s cols per matmul) and the
   small V chunk [k,64] is the stationary weight -> only 2*n_k matmuls per
   sequence, h=0/h=1 col-tiled into one [128, s] PSUM accumulator
   (partitions 0-63 / 64-127 via tile_position auto-derive).
 - output is written transposed ([he, token] image); host untransposes.
 - full 128-row k-chunks live in one partition-major image loaded with ~5
   giant slab DMAs (multi-KB per-partition lines); ragged remainder
   k-chunks live in a second image DMA'd as [kr, 2s] rectangles and
   contracted with K=kr -> zero padding bytes on the wire.
 - per-core HBM traffic ~19.9 MB (PTF 11.5 + PTR 2.7 + V 3.1 + out 2.6).
"""

import math

import numpy as np

import bass_rust
import concourse.bass as bass
import concourse.tile as tile
import concourse.mybir as mybir
from concourse.vector_clock import ScopedClock
from concourse.bass2jax import install_neuronx_cc_hook, _bass_exec_p

# ---------------------------------------------------------------------------
# Workarounds for the in-container walrus build, which only accepts a small
# number of sem waits per instruction: split excess waits onto NoOps placed
# immediately before the instruction on the same engine queue.
# ---------------------------------------------------------------------------
MAX_WAITS = 1

_nop_ctr = [0]


def _mk_wait_nop(engine, waits):
    _nop_ctr[0] += 1
    nop = bass_rust.InstNoOp(name=f"I-waitsplit-{_nop_ctr[0]}", ins=[], outs=[],
                             engine=engine)
    nop.sync_info = bass_rust.SyncInfo(on_wait=list(waits), on_update=[])
    return nop


def _split_inst_waits(ordered):
    for bb_name, insts in ordered.items():
        new = []
        for inst in insts:
            si = getattr(inst, "sync_info", None)
            eng = getattr(inst, "engine", None)
            if si is not None and eng is not None:
                waits = list(si.on_wait)
                if len(waits) > MAX_WAITS:
                    extra, keep = waits[:-MAX_WAITS], waits[-MAX_WAITS:]
                    for j in range(0, len(extra), MAX_WAITS):
                        new.append(_mk_wait_nop(eng, extra[j:j + MAX_WAITS]))
                    inst.sync_info = bass_rust.SyncInfo(
                        on_wait=keep, on_update=list(si.on_update))
            new.append(inst)
        insts[:] = new
    return ordered


if not getattr(tile.TileContext, "_waitsplit_patched", False):
    _orig_lower = tile.TileContext._lower_ordered_insts

    def _patched_lower(self, ordered):
        return _orig_lower(self, _split_inst_waits(ordered))

    def _patched_drain_and_barrier(self, tick_clock, wait_clock):
        nc = self.nc
        drain_inst = nc.sync.drain()
        wait_clock.add_sem_waits(
            drain_inst.ins, ScopedClock({None: tick_clock.global_clock}))
        si = drain_inst.ins.sync_info
        waits = list(si.on_wait)
        if len(waits) > MAX_WAITS:
            drain_inst.ins.sync_info = bass_rust.SyncInfo(
                on_wait=waits[:MAX_WAITS], on_update=list(si.on_update))
            for j in range(MAX_WAITS, len(waits), MAX_WAITS):
                nop = nc.sync.nop(nofuse=True)
                nop.ins.sync_info = bass_rust.SyncInfo(
                    on_wait=waits[j:j + MAX_WAITS], on_update=[])
        nc.all_engine_barrier()
        assert self.sems is not None
        popped = nc._tile_sem_poison_stack.pop()
        assert popped is self._sem_poison
        # leaner clear: sem_clear only (skip the slow gpsimd dma_reset —
        # every DMA has completed by the post-drain barrier above)
        sems = list(self.sems.allocated().values())
        if sems:
            from concourse.bass import SemaphoreHandle, compact_to_ranges
            sem_nums = [s.num if isinstance(s, SemaphoreHandle) else s
                        for s in sems]
            for sem_range in compact_to_ranges(sem_nums):
                assert nc._state.free_isdisjoint(sem_range)
                nc.gpsimd.sem_clear(sem_range)
            nc._state.prepend_free_semaphores(sem_nums)
            for poison_set in nc._tile_sem_poison_stack:
                poison_set.update(sem_nums)
        nc.all_engine_barrier()

    tile.TileContext._lower_ordered_insts = _patched_lower
    tile.TileContext._drain_and_barrier = _patched_drain_and_barrier
    tile.TileContext._waitsplit_patched = True

HEADS = 16
EMBED = 64
BATCH = 32
N_CORES = 8
P = 128  # partitions

SEQS = [128 + 12 * i for i in range(BATCH)]
NTOK = sum(SEQS)  # 10048
_A = np.concatenate([[0], np.cumsum([HEADS * s * s for s in SEQS])])
_B = np.concatenate([[0], np.cumsum(SEQS)])
# schedule: interleave big/small so per-slab DMA+compute mix is uniform
_DESC = sorted(range(BATCH), key=lambda i: -SEQS[i])
ORDER = []
for _j in range(BATCH // 2):
    ORDER.append(_DESC[_j])
    ORDER.append(_DESC[BATCH - 1 - _j])
NF = {i: SEQS[i] // P for i in range(BATCH)}          # full k-chunks
KR = {i: SEQS[i] - NF[i] * P for i in range(BATCH)}    # remainder k rows
NK = {i: NF[i] + (1 if KR[i] else 0) for i in range(BATCH)}

# column layouts of the per-core partition-major images
# PTF (full chunks): per seq 2*nf*s cols; chunk (h, kc<nf) at
#   FOFF + h*nf*s + kc*s, width s (cols = q), row p = k = kc*128+p.
# PTR (remainders): per seq (kr>0) 2*s cols at ROFF; [h0 s][h1 s],
#   rows 0..kr-1 = k = nf*128+p.  Rows kr..127 exist in the host image
#   but are never transferred.
# V: per seq n_k*128 cols; chunk kc at VOFF + kc*128, width 128
#   (= 2 heads x 64), row p = token kc*128+p (zero-padded rows).
# OUT (transposed): per seq s cols at OOFF; partition = he (2*64),
#   col = local token q.
_FOFF = {}
_ROFF = {}
_VOFF = {}
_OOFF = {}
_f = _r = _v = _o = 0
for _i in ORDER:
    _FOFF[_i] = _f
    _ROFF[_i] = _r
    _VOFF[_i] = _v
    _OOFF[_i] = _o
    _f += 2 * NF[_i] * SEQS[_i]
    if KR[_i]:
        _r += 2 * SEQS[_i]
    _v += NK[_i] * P
    _o += SEQS[_i]
F_COLS = _f   # 44976
R_COLS = _r   # 19840
V_COLS = _v   # 12032
O_COLS = _o   # 10048

# slab grouping of consecutive ORDER seqs for the PTF loads / OUT stores
def _make_slabs(targets, cols_of):
    slabs = []
    cur = []
    cur_c = 0
    t = 0
    for i in ORDER:
        c = cols_of(i)
        cur.append(i)
        cur_c += c
        if cur_c >= targets[min(t, len(targets) - 1)]:
            slabs.append(cur)
            cur = []
            cur_c = 0
            t += 1
    if cur:
        slabs.append(cur)
    return slabs


# graded ramp: small first slabs so compute starts early, then steady
PTF_SLABS = _make_slabs([1500, 3000, 4500], lambda i: 2 * NF[i] * SEQS[i])
OUT_SLABS = _make_slabs([1500], lambda i: SEQS[i])
# split the final out slab into per-seq stores so the very last store (after
# the last copy, unoverlappable) is tiny
if len(OUT_SLABS[-1]) > 1:
    OUT_SLABS = OUT_SLABS[:-1] + [[j] for j in OUT_SLABS[-1]]

CDT = mybir.dt.bfloat16
ODT = mybir.dt.bfloat16


def _np_bf16():
    import ml_dtypes

    return ml_dtypes.bfloat16


def build_program(repeat: int = 1):
    """Build the Bass program (one SPMD program shared by all 8 cores)."""
    nc = bass.Bass("TRN2", target_bir_lowering=False, debug=False,
                   num_devices=N_CORES)
    pf_d = nc.dram_tensor("pf", [P, F_COLS], CDT, kind="ExternalInput").ap()
    pr_d = nc.dram_tensor("pr", [P, R_COLS], CDT, kind="ExternalInput").ap()
    v_d = nc.dram_tensor("v", [P, V_COLS], CDT, kind="ExternalInput").ap()
    o_d = nc.dram_tensor("o", [P, O_COLS], ODT, kind="ExternalOutput").ap()

    slab_of = {}
    for t, grp in enumerate(PTF_SLABS):
        for i in grp:
            slab_of[i] = t
    oslab_of = {}
    for t, grp in enumerate(OUT_SLABS):
        for i in grp:
            oslab_of[i] = t

    with tile.TileContext(nc) as tc:
        with (
            tc.tile_pool(name="ptf", bufs=6) as ptf_pool,
            tc.tile_pool(name="ptr", bufs=24) as ptr_pool,
            tc.tile_pool(name="vres", bufs=1) as v_pool,
            tc.tile_pool(name="accp", bufs=8, space="PSUM") as acc_pool,
            tc.tile_pool(name="outsb", bufs=6) as out_pool,
        ):
            for _rep in range(repeat):
                # resident V tile, loaded just-in-time per slab-group so the
                # early wire bandwidth goes to the slabs compute needs first
                vt = v_pool.tile([P, V_COLS], CDT, name="vt", tag="vt")

                slab_tiles = {}
                oslab_tiles = {}
                rem_tiles = {}

                def load_slab(t):
                    grp = PTF_SLABS[t]
                    c0 = _FOFF[grp[0]]
                    cols = sum(2 * NF[j] * SEQS[j] for j in grp)
                    st = ptf_pool.tile([P, cols], CDT, name=f"ptf{t}",
                                       tag="ptf")
                    nc.sync.dma_start(st[:], pf_d[:, c0:c0 + cols])
                    slab_tiles[t] = (st, c0)

                def load_rem(i):
                    s = SEQS[i]
                    kr = KR[i]
                    rt = ptr_pool.tile([kr, 2 * s], CDT, name=f"ptr{i}",
                                       tag="ptr")
                    nc.sync.dma_start(
                        rt[:], pr_d[0:kr, _ROFF[i]:_ROFF[i] + 2 * s])
                    rem_tiles[i] = rt

                # prefetch depth: slabs 0..2, each followed by its rems so
                # the ring delivers a slab's remainders right behind it
                n_slabs = len(PTF_SLABS)

                def load_group(t):
                    load_slab(t)
                    grp = PTF_SLABS[t]
                    vb0 = _VOFF[grp[0]]
                    vb1 = _VOFF[grp[-1]] + NK[grp[-1]] * P
                    nc.sync.dma_start(vt[:, vb0:vb1], v_d[:, vb0:vb1])
                    for i in grp:
                        if KR[i]:
                            load_rem(i)

                for t in range(min(5, n_slabs)):
                    load_group(t)

                flip = 0
                for t, grp in enumerate(PTF_SLABS):
                    st, c0 = slab_tiles[t]
                    if t + 5 < n_slabs:
                        load_group(t + 5)
                    for i in grp:
                        s = SEQS[i]
                        nf = NF[i]
                        kr = KR[i]
                        v0 = _VOFF[i]
                        ot = oslab_of[i]
                        if ot not in oslab_tiles:
                            ogrp = OUT_SLABS[ot]
                            oslab_tiles[ot] = (
                                out_pool.tile([P, sum(SEQS[j] for j in ogrp)],
                                              ODT, name=f"osb{ot}", tag="osb"),
                                _OOFF[ogrp[0]],
                                sum(SEQS[j] for j in ogrp))
                        osb, o0, ocols = oslab_tiles[ot]

                        acc = acc_pool.tile([P, s], mybir.dt.float32,
                                            name=f"acc{i}", tag="acc")
                        # full-chunk matmuls for both heads first (depend
                        # only on the slab), ragged-remainder matmuls last
                        # (depend on the late-arriving rem tile)
                        for h in (0, 1):
                            hoff = _FOFF[i] - c0 + h * nf * s
                            for kc in range(nf):
                                nc.tensor.matmul(
                                    acc[h * EMBED:(h + 1) * EMBED, 0:s],
                                    lhsT=vt[:, v0 + kc * P + h * EMBED:
                                            v0 + kc * P + (h + 1) * EMBED],
                                    rhs=st[:, hoff + kc * s:
                                           hoff + (kc + 1) * s],
                                    start=(kc == 0),
                                    stop=(kc == nf - 1 and not kr),
                                )
                        if kr:
                            rt = rem_tiles[i]
                            for h in (0, 1):
                                nc.tensor.matmul(
                                    acc[h * EMBED:(h + 1) * EMBED, 0:s],
                                    lhsT=vt[0:kr, v0 + nf * P + h * EMBED:
                                            v0 + nf * P + (h + 1) * EMBED],
                                    rhs=rt[0:kr, h * s:(h + 1) * s],
                                    start=(nf == 0),
                                    stop=True,
                                )
                        # PSUM -> SBUF (cast to bf16), alternating engines
                        dst = osb[:, _OOFF[i] - o0:_OOFF[i] - o0 + s]
                        if flip == 0:
                            nc.vector.tensor_copy(dst, acc[:])
                        else:
                            nc.scalar.copy(dst, acc[:])
                        flip ^= 1
                        # if this seq completes its out slab, store it
                        if i == OUT_SLABS[ot][-1]:
                            nc.scalar.dma_start(o_d[:, o0:o0 + ocols], osb[:])
                            del oslab_tiles[ot]
    return nc


def pack_inputs(batch1: np.ndarray, batch2: np.ndarray):
    """Build per-core packed (ptf, ptr, v) host buffers (bf16 images)."""
    bf16 = _np_bf16()
    b2 = np.ascontiguousarray(batch2).reshape(NTOK, HEADS * EMBED)
    cores = []
    for c in range(N_CORES):
        fimg = np.zeros((P, F_COLS), dtype=bf16)
        rimg = np.zeros((P, R_COLS), dtype=bf16)
        vimg = np.zeros((P, V_COLS), dtype=bf16)
        for i in ORDER:
            s = SEQS[i]
            nf = NF[i]
            kr = KR[i]
            n_k = NK[i]
            blk = batch1[_A[i] + 2 * c * s * s:
                         _A[i] + (2 * c + 2) * s * s].reshape(2, s, s)
            pt = np.ascontiguousarray(blk.transpose(0, 2, 1))  # [h, k, q]
            full = pt[:, :nf * P, :].reshape(2, nf, P, s)
            full = full.transpose(2, 0, 1, 3).reshape(P, 2 * nf * s)
            fimg[:, _FOFF[i]:_FOFF[i] + 2 * nf * s] = full.astype(bf16)
            if kr:
                rem = pt[:, nf * P:s, :]                      # [2, kr, s]
                rem = rem.transpose(1, 0, 2).reshape(kr, 2 * s)
                rimg[0:kr, _ROFF[i]:_ROFF[i] + 2 * s] = rem.astype(bf16)

            kpad = n_k * P
            vv = np.zeros((kpad, P), dtype=np.float32)
            vv[:s] = b2[_B[i]:_B[i] + s, 2 * c * EMBED:(2 * c + 2) * EMBED]
            vv = vv.reshape(n_k, P, P).transpose(1, 0, 2).reshape(P, n_k * P)
            vimg[:, _VOFF[i]:_VOFF[i] + n_k * P] = vv.astype(bf16)
        cores.append({"pf": fimg, "pr": rimg, "v": vimg})
    return cores


def unpack_outputs(o_cores) -> np.ndarray:
    """Scatter per-core transposed outputs back to [NTOK, HEADS, EMBED]."""
    out = np.empty((NTOK, HEADS * EMBED), dtype=np.float32)
    for c in range(N_CORES):
        oc = np.asarray(o_cores[c])
        for i in ORDER:
            s = SEQS[i]
            blk = oc[:, _OOFF[i]:_OOFF[i] + s]     # [he, q]
            out[_B[i]:_B[i] + s,
                2 * c * EMBED:(2 * c + 2) * EMBED] = blk.T.astype(np.float32)
    return out.reshape(NTOK, HEADS, EMBED)


# ---------------------------------------------------------------------------
# Execution: cached jitted shard_map over 8 cores (axon/PJRT path).
# ---------------------------------------------------------------------------
_CACHE = {}


def run_packed(core_inputs):
    """Run the SPMD program; returns list of per-core packed outputs."""
    import concourse.bass_utils as bass_utils

    if ("nc", 1) not in _CACHE:
        _CACHE[("nc", 1)] = build_program()
    nc = _CACHE[("nc", 1)]
    res = bass_utils.run_bass_kernel_spmd(nc, core_inputs,
                                          core_ids=list(range(N_CORES)))
    return [res.results[c]["o"] for c in range(N_CORES)]


def kernel(batch1, batch2, batch, seqlen) -> np.ndarray:
    batch1 = np.asarray(batch1, dtype=np.float32)
    batch2 = np.asarray(batch2, dtype=np.float32)
    core_inputs = pack_inputs(batch1, batch2)
    o_cores = run_packed(core_inputs)
    return unpack_outputs(o_cores)


# revision 25
# speedup vs baseline: 1.0267x; 1.0267x over previous
"""Trainium2 Bass kernel for ragged bmm2 (attention probs @ V, grouped GEMM).

Problem: 32 ragged sequences, lengths s_i = 128 + 12*i (128..500), 16 heads,
embed 64.  batch1 = packed per-(seq,head) [s,s] prob blocks (fp32, ~227MB),
batch2 = packed V [ntokens, 16*64].  out[q,h,e] = sum_k P[h,q,k] V[k,h,e].

Sharding: head-parallel.  Core c handles heads (2c, 2c+1) for ALL sequences.

v3 design (memory-roofline oriented):
 - host pre-transposes P into PT[k, q] (bf16) so the device does no
   transposes; PT is the *moving* operand (N=s cols per matmul) and the
   small V chunk [k,64] is the stationary weight -> only 2*n_k matmuls per
   sequence, h=0/h=1 col-tiled into one [128, s] PSUM accumulator
   (partitions 0-63 / 64-127 via tile_position auto-derive).
 - output is written transposed ([he, token] image); host untransposes.
 - full 128-row k-chunks live in one partition-major image loaded with ~5
   giant slab DMAs (multi-KB per-partition lines); ragged remainder
   k-chunks live in a second image DMA'd as [kr, 2s] rectangles and
   contracted with K=kr -> zero padding bytes on the wire.
 - per-core HBM traffic ~19.9 MB (PTF 11.5 + PTR 2.7 + V 3.1 + out 2.6).
"""

import math

import numpy as np

import bass_rust
import concourse.bass as bass
import concourse.tile as tile
import concourse.mybir as mybir
from concourse.vector_clock import ScopedClock
from concourse.bass2jax import install_neuronx_cc_hook, _bass_exec_p

# ---------------------------------------------------------------------------
# Workarounds for the in-container walrus build, which only accepts a small
# number of sem waits per instruction: split excess waits onto NoOps placed
# immediately before the instruction on the same engine queue.
# ---------------------------------------------------------------------------
MAX_WAITS = 1

_nop_ctr = [0]


def _mk_wait_nop(engine, waits):
    _nop_ctr[0] += 1
    nop = bass_rust.InstNoOp(name=f"I-waitsplit-{_nop_ctr[0]}", ins=[], outs=[],
                             engine=engine)
    nop.sync_info = bass_rust.SyncInfo(on_wait=list(waits), on_update=[])
    return nop


def _split_inst_waits(ordered):
    for bb_name, insts in ordered.items():
        new = []
        for inst in insts:
            si = getattr(inst, "sync_info", None)
            eng = getattr(inst, "engine", None)
            if si is not None and eng is not None:
                waits = list(si.on_wait)
                if len(waits) > MAX_WAITS:
                    extra, keep = waits[:-MAX_WAITS], waits[-MAX_WAITS:]
                    for j in range(0, len(extra), MAX_WAITS):
                        new.append(_mk_wait_nop(eng, extra[j:j + MAX_WAITS]))
                    inst.sync_info = bass_rust.SyncInfo(
                        on_wait=keep, on_update=list(si.on_update))
            new.append(inst)
        insts[:] = new
    return ordered


if not getattr(tile.TileContext, "_waitsplit_patched", False):
    _orig_lower = tile.TileContext._lower_ordered_insts

    def _patched_lower(self, ordered):
        return _orig_lower(self, _split_inst_waits(ordered))

    def _patched_drain_and_barrier(self, tick_clock, wait_clock):
        nc = self.nc
        drain_inst = nc.sync.drain()
        wait_clock.add_sem_waits(
            drain_inst.ins, ScopedClock({None: tick_clock.global_clock}))
        si = drain_inst.ins.sync_info
        waits = list(si.on_wait)
        if len(waits) > MAX_WAITS:
            drain_inst.ins.sync_info = bass_rust.SyncInfo(
                on_wait=waits[:MAX_WAITS], on_update=list(si.on_update))
            for j in range(MAX_WAITS, len(waits), MAX_WAITS):
                nop = nc.sync.nop(nofuse=True)
                nop.ins.sync_info = bass_rust.SyncInfo(
                    on_wait=waits[j:j + MAX_WAITS], on_update=[])
        nc.all_engine_barrier()
        assert self.sems is not None
        popped = nc._tile_sem_poison_stack.pop()
        assert popped is self._sem_poison
        # leaner clear: sem_clear only (skip the slow gpsimd dma_reset —
        # every DMA has completed by the post-drain barrier above)
        sems = list(self.sems.allocated().values())
        if sems:
            from concourse.bass import SemaphoreHandle, compact_to_ranges
            sem_nums = [s.num if isinstance(s, SemaphoreHandle) else s
                        for s in sems]
            for sem_range in compact_to_ranges(sem_nums):
                assert nc._state.free_isdisjoint(sem_range)
                nc.gpsimd.sem_clear(sem_range)
            nc._state.prepend_free_semaphores(sem_nums)
            for poison_set in nc._tile_sem_poison_stack:
                poison_set.update(sem_nums)
        nc.all_engine_barrier()

    tile.TileContext._lower_ordered_insts = _patched_lower
    tile.TileContext._drain_and_barrier = _patched_drain_and_barrier
    tile.TileContext._waitsplit_patched = True

HEADS = 16
EMBED = 64
BATCH = 32
N_CORES = 8
P = 128  # partitions

SEQS = [128 + 12 * i for i in range(BATCH)]
NTOK = sum(SEQS)  # 10048
_A = np.concatenate([[0], np.cumsum([HEADS * s * s for s in SEQS])])
_B = np.concatenate([[0], np.cumsum(SEQS)])
# schedule: ascending length — tiny seqs first (pipeline ramps while the
# prefetch stream fills), big dense seqs last (PE stays warm, best DMA
# efficiency when the pipeline is deepest)
ORDER = sorted(range(BATCH), key=lambda i: SEQS[i])
NF = {i: SEQS[i] // P for i in range(BATCH)}          # full k-chunks
KR = {i: SEQS[i] - NF[i] * P for i in range(BATCH)}    # remainder k rows
NK = {i: NF[i] + (1 if KR[i] else 0) for i in range(BATCH)}

# column layouts of the per-core partition-major images
# PTF (full chunks): per seq 2*nf*s cols; chunk (h, kc<nf) at
#   FOFF + h*nf*s + kc*s, width s (cols = q), row p = k = kc*128+p.
# PTR (remainders): per seq (kr>0) 2*s cols at ROFF; [h0 s][h1 s],
#   rows 0..kr-1 = k = nf*128+p.  Rows kr..127 exist in the host image
#   but are never transferred.
# V: per seq n_k*128 cols; chunk kc at VOFF + kc*128, width 128
#   (= 2 heads x 64), row p = token kc*128+p (zero-padded rows).
# OUT (transposed): per seq s cols at OOFF; partition = he (2*64),
#   col = local token q.
_FOFF = {}
_ROFF = {}
_VOFF = {}
_OOFF = {}
_f = _r = _v = _o = 0
for _i in ORDER:
    _FOFF[_i] = _f
    _ROFF[_i] = _r
    _VOFF[_i] = _v
    _OOFF[_i] = _o
    _f += 2 * NF[_i] * SEQS[_i]
    if KR[_i]:
        _r += 2 * SEQS[_i]
    _v += NK[_i] * P
    _o += SEQS[_i]
F_COLS = _f   # 44976
R_COLS = _r   # 19840
V_COLS = _v   # 12032
O_COLS = _o   # 10048

# slab grouping of consecutive ORDER seqs for the PTF loads / OUT stores
def _make_slabs(targets, cols_of):
    slabs = []
    cur = []
    cur_c = 0
    t = 0
    for i in ORDER:
        c = cols_of(i)
        cur.append(i)
        cur_c += c
        if cur_c >= targets[min(t, len(targets) - 1)]:
            slabs.append(cur)
            cur = []
            cur_c = 0
            t += 1
    if cur:
        slabs.append(cur)
    return slabs


# graded ramp: small first slabs so compute starts early, then steady
PTF_SLABS = _make_slabs([1500, 3000, 4500], lambda i: 2 * NF[i] * SEQS[i])
OUT_SLABS = _make_slabs([1500], lambda i: SEQS[i])
# split the final out slab into per-seq stores so the very last store (after
# the last copy, unoverlappable) is tiny
if len(OUT_SLABS[-1]) > 1:
    OUT_SLABS = OUT_SLABS[:-1] + [[j] for j in OUT_SLABS[-1]]

CDT = mybir.dt.bfloat16
ODT = mybir.dt.bfloat16


def _np_bf16():
    import ml_dtypes

    return ml_dtypes.bfloat16


def build_program(repeat: int = 1):
    """Build the Bass program (one SPMD program shared by all 8 cores)."""
    nc = bass.Bass("TRN2", target_bir_lowering=False, debug=False,
                   num_devices=N_CORES)
    pf_d = nc.dram_tensor("pf", [P, F_COLS], CDT, kind="ExternalInput").ap()
    pr_d = nc.dram_tensor("pr", [P, R_COLS], CDT, kind="ExternalInput").ap()
    v_d = nc.dram_tensor("v", [P, V_COLS], CDT, kind="ExternalInput").ap()
    o_d = nc.dram_tensor("o", [P, O_COLS], ODT, kind="ExternalOutput").ap()

    slab_of = {}
    for t, grp in enumerate(PTF_SLABS):
        for i in grp:
            slab_of[i] = t
    oslab_of = {}
    for t, grp in enumerate(OUT_SLABS):
        for i in grp:
            oslab_of[i] = t

    with tile.TileContext(nc) as tc:
        with (
            tc.tile_pool(name="ptf", bufs=6) as ptf_pool,
            tc.tile_pool(name="ptr", bufs=24) as ptr_pool,
            tc.tile_pool(name="vres", bufs=1) as v_pool,
            tc.tile_pool(name="accp", bufs=8, space="PSUM") as acc_pool,
            tc.tile_pool(name="outsb", bufs=6) as out_pool,
        ):
            for _rep in range(repeat):
                # resident V tile, loaded just-in-time per slab-group so the
                # early wire bandwidth goes to the slabs compute needs first
                vt = v_pool.tile([P, V_COLS], CDT, name="vt", tag="vt")

                slab_tiles = {}
                oslab_tiles = {}
                rem_tiles = {}

                def load_slab(t):
                    grp = PTF_SLABS[t]
                    c0 = _FOFF[grp[0]]
                    cols = sum(2 * NF[j] * SEQS[j] for j in grp)
                    st = ptf_pool.tile([P, cols], CDT, name=f"ptf{t}",
                                       tag="ptf")
                    nc.sync.dma_start(st[:], pf_d[:, c0:c0 + cols])
                    slab_tiles[t] = (st, c0)

                def load_rem(i):
                    s = SEQS[i]
                    kr = KR[i]
                    rt = ptr_pool.tile([kr, 2 * s], CDT, name=f"ptr{i}",
                                       tag="ptr")
                    nc.sync.dma_start(
                        rt[:], pr_d[0:kr, _ROFF[i]:_ROFF[i] + 2 * s])
                    rem_tiles[i] = rt

                # prefetch depth: slabs 0..2, each followed by its rems so
                # the ring delivers a slab's remainders right behind it
                n_slabs = len(PTF_SLABS)

                def load_group(t):
                    load_slab(t)
                    grp = PTF_SLABS[t]
                    vb0 = _VOFF[grp[0]]
                    vb1 = _VOFF[grp[-1]] + NK[grp[-1]] * P
                    nc.sync.dma_start(vt[:, vb0:vb1], v_d[:, vb0:vb1])
                    for i in grp:
                        if KR[i]:
                            load_rem(i)

                for t in range(min(5, n_slabs)):
                    load_group(t)

                flip = 0
                for t, grp in enumerate(PTF_SLABS):
                    st, c0 = slab_tiles[t]
                    if t + 5 < n_slabs:
                        load_group(t + 5)
                    for i in grp:
                        s = SEQS[i]
                        nf = NF[i]
                        kr = KR[i]
                        v0 = _VOFF[i]
                        ot = oslab_of[i]
                        if ot not in oslab_tiles:
                            ogrp = OUT_SLABS[ot]
                            oslab_tiles[ot] = (
                                out_pool.tile([P, sum(SEQS[j] for j in ogrp)],
                                              ODT, name=f"osb{ot}", tag="osb"),
                                _OOFF[ogrp[0]],
                                sum(SEQS[j] for j in ogrp))
                        osb, o0, ocols = oslab_tiles[ot]

                        acc = acc_pool.tile([P, s], mybir.dt.float32,
                                            name=f"acc{i}", tag="acc")
                        # full-chunk matmuls for both heads first (depend
                        # only on the slab), ragged-remainder matmuls last
                        # (depend on the late-arriving rem tile)
                        for h in (0, 1):
                            hoff = _FOFF[i] - c0 + h * nf * s
                            for kc in range(nf):
                                nc.tensor.matmul(
                                    acc[h * EMBED:(h + 1) * EMBED, 0:s],
                                    lhsT=vt[:, v0 + kc * P + h * EMBED:
                                            v0 + kc * P + (h + 1) * EMBED],
                                    rhs=st[:, hoff + kc * s:
                                           hoff + (kc + 1) * s],
                                    start=(kc == 0),
                                    stop=(kc == nf - 1 and not kr),
                                )
                        if kr:
                            rt = rem_tiles[i]
                            for h in (0, 1):
                                nc.tensor.matmul(
                                    acc[h * EMBED:(h + 1) * EMBED, 0:s],
                                    lhsT=vt[0:kr, v0 + nf * P + h * EMBED:
                                            v0 + nf * P + (h + 1) * EMBED],
                                    rhs=rt[0:kr, h * s:(h + 1) * s],
                                    start=(nf == 0),
                                    stop=True,
                                )
                        # PSUM -> SBUF (cast to bf16), alternating engines
                        dst = osb[:, _OOFF[i] - o0:_OOFF[i] - o0 + s]
                        if flip == 0:
                            nc.vector.tensor_copy(dst, acc[:])
                        else:
                            nc.scalar.copy(dst, acc[:])
                        flip ^= 1
                        # if this seq completes its out slab, store it
                        if i == OUT_SLABS[ot][-1]:
                            nc.scalar.dma_start(o_d[:, o0:o0 + ocols], osb[:])
                            del oslab_tiles[ot]
    return nc


def pack_inputs(batch1: np.ndarray, batch2: np.ndarray):
    """Build per-core packed (ptf, ptr, v) host buffers (bf16 images)."""
    bf16 = _np_bf16()
    b2 = np.ascontiguousarray(batch2).reshape(NTOK, HEADS * EMBED)
    cores = []
    for c in range(N_CORES):
        fimg = np.zeros((P, F_COLS), dtype=bf16)
        rimg = np.zeros((P, R_COLS), dtype=bf16)
        vimg = np.zeros((P, V_COLS), dtype=bf16)
        for i in ORDER:
            s = SEQS[i]
            nf = NF[i]
            kr = KR[i]
            n_k = NK[i]
            blk = batch1[_A[i] + 2 * c * s * s:
                         _A[i] + (2 * c + 2) * s * s].reshape(2, s, s)
            pt = np.ascontiguousarray(blk.transpose(0, 2, 1))  # [h, k, q]
            full = pt[:, :nf * P, :].reshape(2, nf, P, s)
            full = full.transpose(2, 0, 1, 3).reshape(P, 2 * nf * s)
            fimg[:, _FOFF[i]:_FOFF[i] + 2 * nf * s] = full.astype(bf16)
            if kr:
                rem = pt[:, nf * P:s, :]                      # [2, kr, s]
                rem = rem.transpose(1, 0, 2).reshape(kr, 2 * s)
                rimg[0:kr, _ROFF[i]:_ROFF[i] + 2 * s] = rem.astype(bf16)

            kpad = n_k * P
            vv = np.zeros((kpad, P), dtype=np.float32)
            vv[:s] = b2[_B[i]:_B[i] + s, 2 * c * EMBED:(2 * c + 2) * EMBED]
            vv = vv.reshape(n_k, P, P).transpose(1, 0, 2).reshape(P, n_k * P)
            vimg[:, _VOFF[i]:_VOFF[i] + n_k * P] = vv.astype(bf16)
        cores.append({"pf": fimg, "pr": rimg, "v": vimg})
    return cores


def unpack_outputs(o_cores) -> np.ndarray:
    """Scatter per-core transposed outputs back to [NTOK, HEADS, EMBED]."""
    out = np.empty((NTOK, HEADS * EMBED), dtype=np.float32)
    for c in range(N_CORES):
        oc = np.asarray(o_cores[c])
        for i in ORDER:
            s = SEQS[i]
            blk = oc[:, _OOFF[i]:_OOFF[i] + s]     # [he, q]
            out[_B[i]:_B[i] + s,
                2 * c * EMBED:(2 * c + 2) * EMBED] = blk.T.astype(np.float32)
    return out.reshape(NTOK, HEADS, EMBED)


# ---------------------------------------------------------------------------
# Execution: cached jitted shard_map over 8 cores (axon/PJRT path).
# ---------------------------------------------------------------------------
_CACHE = {}


def run_packed(core_inputs):
    """Run the SPMD program; returns list of per-core packed outputs."""
    import concourse.bass_utils as bass_utils

    if ("nc", 1) not in _CACHE:
        _CACHE[("nc", 1)] = build_program()
    nc = _CACHE[("nc", 1)]
    res = bass_utils.run_bass_kernel_spmd(nc, core_inputs,
                                          core_ids=list(range(N_CORES)))
    return [res.results[c]["o"] for c in range(N_CORES)]


def kernel(batch1, batch2, batch, seqlen) -> np.ndarray:
    batch1 = np.asarray(batch1, dtype=np.float32)
    batch2 = np.asarray(batch2, dtype=np.float32)
    core_inputs = pack_inputs(batch1, batch2)
    o_cores = run_packed(core_inputs)
    return unpack_outputs(o_cores)
